# revision 2
# baseline (speedup 1.0000x reference)
"""Multi-head attention (RoPE, causal) Trainium2 kernel, 8-way sharded. v2.

Sharding: core c -> (batch b = c//2, head-group g = c%2 of 8 heads).
Each core computes its batch/head-group's attention output projected through
its W_proj row-slice; the host sums the two partial projections per batch
(fp16 partials, fp32 accumulate) and adds b_proj.

v2 changes vs baseline (all per-core):
  - exp fused across 2-ktile groups (4-bank PSUM pst) -> half the ACT
    instructions; diag groups run at reduced width.
  - S^T and P^T@V matmuls column-restricted on diagonal tiles.
  - causal mask = in-place multiply by a single [128,2,128] triangle on Pool
    (gpsimd), off ACT/DVE; no memsets (pst pre-zeroed once).
  - rope PSUM evacuation on DVE (plain copy; qkv bias folded in only when
    nonzero), v evacuation on DVE.
  - softmax reciprocal reads PSUM directly (no staging copy).
  - projection output staged to fp16, host sums partials in fp32.
  - software-pipelined emission: qk GEMMs for pair jj+1 and output-projection
    tiles are split into small "filler" units pumped into the PE queue
    between attention groups so the PE never idles while ACT runs exp.
"""

import numpy as np

import concourse.bass as bass
import concourse.bacc as bacc
import concourse.tile as tile
import concourse.mybir as mybir
from concourse.bass_utils import run_bass_kernel_spmd

F16 = mybir.dt.float16
F32 = mybir.dt.float32

B, L, D = 4, 2048, 1024
H, Dh = 16, 64
ROPE_THETA = 10000.0
N_CORES = 8
HL = 8           # heads per core
DC = D // 128    # 8 contraction chunks
NJJ = 4          # head pairs per core
NTC = L // 512   # 4 token chunks of 512
NTT = L // 128   # 16 token tiles of 128
NQC = L // 512   # 4 q chunks of 512


MM_LABELS = []


def _emit(nc, tc, dram, use_bias, debug=False):
    from contextlib import ExitStack

    _raw_mm = nc.tensor.matmul

    def _mm(label):
        MM_LABELS.append(label)
        return _raw_mm

    with ExitStack() as ctx:
        consts = ctx.enter_context(tc.tile_pool(name="consts", bufs=1))
        rope = ctx.enter_context(tc.tile_pool(name="rope", bufs=2))
        ptp = ctx.enter_context(tc.tile_pool(name="ptp", bufs=4))
        small = ctx.enter_context(tc.tile_pool(name="small", bufs=2))
        ostg = ctx.enter_context(tc.tile_pool(name="ostg", bufs=4))

        # ---- resident tensors -------------------------------------------
        xT = consts.tile([128, DC, L], F16)
        wq = consts.tile([128, DC, 512], F16)
        wk = consts.tile([128, DC, 512], F16)
        wv = consts.tile([128, DC, 512], F16)
        wp = consts.tile([128, NJJ, 1024], F16)
        cos4 = consts.tile([128, L], F16)
        sin4 = consts.tile([128, L], F16)
        tri2 = consts.tile([128, 2, 128], F16)
        qT = consts.tile([128, NJJ, L], F16)
        kT = consts.tile([128, NJJ, L], F16)
        vaug = consts.tile([128, NTT, HL, 128], F16)
        oT = consts.tile([128, NJJ, L], F16)
        if use_bias:
            bq = consts.tile([128, NJJ], F32)
            bk = consts.tile([128, NJJ], F32)
            bv = consts.tile([1, 512], F16)
            ones1 = consts.tile([1, 128], F16)
            nc.sync.dma_start(bq[:], dram["bq"].ap())
            nc.sync.dma_start(bk[:], dram["bk"].ap())
            nc.sync.dma_start(bv[:], dram["bv"].ap())
            nc.vector.memset(ones1[:], 1.0)

        for dc in range(DC):
            nc.sync.dma_start(wq[:, dc, :], dram["wq"].ap()[:, dc, :])
            nc.sync.dma_start(wk[:, dc, :], dram["wk"].ap()[:, dc, :])
            nc.sync.dma_start(wv[:, dc, :], dram["wv"].ap()[:, dc, :])
            nc.sync.dma_start(xT[:, dc, :], dram["xT"].ap()[:, dc, :])
        nc.sync.dma_start(cos4[:], dram["cos4"].ap())
        nc.sync.dma_start(sin4[:], dram["sin4"].ap())
        nc.sync.dma_start(tri2[:], dram["tri2"].ap())
        nc.sync.dma_start(wp[:], dram["wp"].ap())
        nc.gpsimd.memset(vaug[:, :, :, 64:128], 1.0)

        # ---- rope: psum -> qT/kT slice ----------------------------------
        def rope_store(ps, dstT, jj, ts, b_sb=None, bj=0):
            raw = rope.tile([128, 512], F16, tag="raw", name=f"raw_{jj}_{ts}")
            if b_sb is not None:
                nc.scalar.activation(
                    raw[:], ps[:], mybir.ActivationFunctionType.Identity,
                    bias=b_sb[:, bj:bj + 1],
                )
            else:
                nc.vector.tensor_copy(raw[:], ps[:])
            swp = rope.tile([128, 512], F16, tag="swp", name=f"swp_{jj}_{ts}")
            for blk in range(4):
                sb_ = (blk ^ 1) * 32
                nc.sync.dma_start(
                    swp[blk * 32:(blk + 1) * 32, :], raw[sb_:sb_ + 32, :]
                )
            t1 = rope.tile([128, 512], F16, tag="t1", name=f"t1_{jj}_{ts}")
            nc.vector.tensor_mul(t1[:], raw[:], cos4[:, ts:ts + 512])
            t2 = rope.tile([128, 512], F16, tag="t2", name=f"t2_{jj}_{ts}")
            nc.vector.tensor_mul(t2[:], swp[:], sin4[:, ts:ts + 512])
            nc.vector.tensor_add(dstT[:, jj, ts:ts + 512], t1[:], t2[:])

        with (
            tc.tile_pool(name="pgm", bufs=2, space="PSUM") as pgm,
        ):
            # ---- phase A: qk pair 0 + v, stream-consuming xT chunks ----
            def v_tile(tt, ps):
                for dc in range(DC):
                    _mm("A:v")(
                        ps[:],
                        xT[:, dc, tt * 128:(tt + 1) * 128],
                        wv[:, dc, :],
                        start=(dc == 0),
                        stop=(dc == DC - 1 and not use_bias),
                    )
                if use_bias:
                    _mm("A:vb")(
                        ps[:], ones1[:], bv[:], start=False, stop=True,
                    )

            def v_evac(tt, ps):
                nc.scalar.copy(
                    vaug[:, tt, :, 0:64],
                    ps[:].rearrange("p (h d) -> p h d", h=HL),
                )

            with tc.tile_pool(name="pga", bufs=1, space="PSUM") as pga, \
                 tc.tile_pool(name="pgk", bufs=1, space="PSUM") as pgk:
                psq = [pga.tile([128, 512], F32, tag=f"g{t}", name=f"q_{t}")
                       for t in range(NTC)]
                psk = [pgk.tile([128, 512], F32, tag=f"k{t}", name=f"k_{t}")
                       for t in range(2)]
                psv = [pgm.tile([128, 512], F32, tag="gemm", name=f"vps_{t}")
                       for t in range(2)]
                for dc in range(DC):
                    for tcn in range(NTC):
                        _mm("A:qk0")(
                            psq[tcn][:], wq[:, dc, 0:128],
                            xT[:, dc, tcn * 512:tcn * 512 + 512],
                            start=(dc == 0), stop=(dc == DC - 1),
                        )
                    for tcn in range(2):
                        _mm("A:qk0")(
                            psk[tcn][:], wk[:, dc, 0:128],
                            xT[:, dc, tcn * 512:tcn * 512 + 512],
                            start=(dc == 0), stop=(dc == DC - 1),
                        )
                    for vt in range(2):
                        _mm("A:v")(
                            psv[vt][:],
                            xT[:, dc, vt * 128:(vt + 1) * 128],
                            wv[:, dc, :],
                            start=(dc == 0),
                            stop=(dc == DC - 1 and not use_bias),
                        )
                bqs = bq if use_bias else None
                bks = bk if use_bias else None
                for tcn in range(NTC):
                    rope_store(psq[tcn], qT, 0, tcn * 512, b_sb=bqs, bj=0)
                if use_bias:
                    for vt in range(2):
                        _mm("A:vb")(
                            psv[vt][:], ones1[:], bv[:],
                            start=False, stop=True,
                        )
                for vt in range(2):
                    v_evac(vt, psv[vt])
                # k tcn 2,3 (xT fully resident now) into freed q banks
                for tcn in (2, 3):
                    ps = pga.tile([128, 512], F32, tag=f"g{tcn}",
                                  name=f"k_{tcn}")
                    for dc in range(DC):
                        _mm("A:qk0")(
                            ps[:], wk[:, dc, 0:128],
                            xT[:, dc, tcn * 512:tcn * 512 + 512],
                            start=(dc == 0), stop=(dc == DC - 1),
                        )
                    rope_store(ps, kT, 0, tcn * 512, b_sb=bks, bj=0)
                for tcn in range(2):
                    rope_store(psk[tcn], kT, 0, tcn * 512, b_sb=bks, bj=0)

                for tt in range(2, NTT):
                    ps = pgm.tile([128, 512], F32, tag="gemm",
                                  name=f"vps_{tt}")
                    v_tile(tt, ps)
                    v_evac(tt, ps)

            # ---- fillers ------------------------------------------------
            fillers = []
            POOLS = {"proj": pgm}

            def make_qk_units(jj):
                units = []
                for side, (w_sb, dstT) in enumerate(((wq, qT), (wk, kT))):
                    for tcn in range(NTC):
                        st = {}

                        def unit(k, w_sb=w_sb, dstT=dstT, tcn=tcn, st=st,
                                 side=side):
                            if k == 0:
                                st["ps"] = pgm.tile(
                                    [128, 512], F32, tag="gemm",
                                    name=f"qkps_{jj}_{side}_{tcn}")
                            ps = st["ps"]
                            for dc in (2 * k, 2 * k + 1):
                                _mm(f"F:qk{jj}")(
                                    ps[:],
                                    w_sb[:, dc, jj * 128:(jj + 1) * 128],
                                    xT[:, dc, tcn * 512:tcn * 512 + 512],
                                    start=(dc == 0),
                                    stop=(dc == DC - 1),
                                )
                            if k == 3:
                                b_sb = ((bq if side == 0 else bk)
                                        if use_bias else None)
                                rope_store(ps, dstT, jj, tcn * 512,
                                           b_sb=b_sb, bj=jj)

                        for k in range(4):
                            units.append((unit, k))
                return units

            def make_proj_units(qc):
                units = []
                for tt in range(4 * qc, 4 * qc + 4):
                    for cc in range(2):
                        def unit(k, tt=tt, cc=cc):
                            ps = POOLS["proj"].tile(
                                [128, 512], F32, tag="gemm",
                                name=f"projps_{tt}_{cc}")
                            for jj in range(NJJ):
                                _mm("F:proj")(
                                    ps[:],
                                    oT[:, jj, tt * 128:(tt + 1) * 128],
                                    wp[:, jj, cc * 512:cc * 512 + 512],
                                    start=(jj == 0), stop=(jj == NJJ - 1),
                                )
                            stage = ostg.tile([128, 512], F16, tag="stg",
                                              name=f"stg_{tt}_{cc}")
                            nc.vector.tensor_copy(stage[:], ps[:])
                            nc.sync.dma_start(
                                dram["out"].ap()[tt * 128:(tt + 1) * 128,
                                                 cc * 512:cc * 512 + 512],
                                stage[:],
                            )
                        units.append((unit, 0))
                return units

            def pump(n):
                for _ in range(n):
                    if not fillers:
                        return
                    f, k = fillers.pop(0)
                    f(k)

            # ---- attention ---------------------------------------------
            with (
                tc.tile_pool(name="pstp", bufs=2, space="PSUM") as pstp,
                tc.tile_pool(name="psop", bufs=1, space="PSUM") as psop,
            ):
                def emit_pv(batch, qc, jj, pso_t):
                    nkt = 4 * qc + 4
                    for ptile, kt in batch:
                        d = kt - 4 * qc
                        c0 = 128 * d if d > 0 else 0
                        for h in range(2):
                            _mm(f"PV:{jj}.{qc}")(
                                pso_t[:, h, c0:512],
                                vaug[:, kt, 2 * jj + h, :],
                                ptile[:, h, c0:512],
                                start=(kt == 0), stop=(kt == nkt - 1),
                                skip_group_check=True,
                            )

                pending_norm = [None]

                def normalize(jj, qc, pso_t):
                    qs = qc * 512
                    for h in range(2):
                        den = small.tile([64, 512], F32, tag="den",
                                         name=f"den_{jj}_{qc}_{h}")
                        nc.vector.tensor_copy(den[:], pso_t[64:128, h, :])
                        rec = small.tile([64, 512], F32, tag="rec",
                                         name=f"rec_{jj}_{qc}_{h}")
                        nc.vector.reciprocal_approx_fast(rec[:], den[:])
                        nc.vector.tensor_mul(
                            oT[64 * h:64 * h + 64, jj, qs:qs + 512],
                            pso_t[0:64, h, :], rec[:],
                        )
                    if jj == NJJ - 1:
                        fillers.extend(make_proj_units(qc))

                for jj in range(NJJ):
                    if jj < NJJ - 1:
                        fillers.extend(make_qk_units(jj + 1))
                    for qc in range(NQC):
                        qs = qc * 512
                        nbat = 2 * qc + 2
                        pso_t = psop.tile([128, 2, 512], F32, tag="pso",
                                          name=f"pso_{jj}_{qc}")
                        prevb = None
                        for g in range(nbat):
                            kts = (2 * g, 2 * g + 1)
                            psts = []
                            # both kts' S pairs back-to-back: one 64-row
                            # mode stretch on the PE
                            for kt in kts:
                                d = kt - 4 * qc
                                c0 = 128 * d if d > 0 else 0
                                ks = kt * 128
                                pst_t = pstp.tile(
                                    [128, 2, 512], F32, tag="pst",
                                    name=f"pst_{jj}_{qc}_{kt}")
                                for h in range(2):
                                    _mm(f"S:{jj}.{qc}")(
                                        pst_t[:, h, c0:512],
                                        kT[64 * h:64 * h + 64, jj,
                                           ks:ks + 128],
                                        qT[64 * h:64 * h + 64, jj,
                                           qs + c0:qs + 512],
                                        start=True, stop=True,
                                    )
                                psts.append((pst_t, kt, c0))
                            batch = []
                            for pst_t, kt, c0 in psts:
                                d = kt - 4 * qc
                                ptile = ptp.tile(
                                    [128, 2, 512], F16, tag="pt",
                                    name=f"pt_{jj}_{qc}_{kt}")
                                nc.scalar.activation(
                                    ptile[:, :, c0:512],
                                    pst_t[:, :, c0:512],
                                    mybir.ActivationFunctionType.Exp,
                                    scale=0.125,
                                )
                                if d >= 0:
                                    nc.gpsimd.tensor_mul(
                                        ptile[:, :, c0:c0 + 128],
                                        ptile[:, :, c0:c0 + 128],
                                        tri2[:],
                                    )
                                batch.append((ptile, kt))
                            if prevb is not None:
                                emit_pv(prevb, qc, jj, pso_t)
                            prevb = batch
                            pump(1)
                            if g == 0 and pending_norm[0] is not None:
                                normalize(*pending_norm[0])
                                pending_norm[0] = None
                        emit_pv(prevb, qc, jj, pso_t)
                        # normalize deferred one batch: keeps the DVE queue
                        # free of PV-dependent work ahead of filler evacs
                        pending_norm[0] = (jj, qc, pso_t)
                        pump(2)
                    # drain enough that next pair's qT/kT are ready
                    if jj < NJJ - 1:
                        pump(len(fillers))
                if pending_norm[0] is not None:
                    normalize(*pending_norm[0])
                    pending_norm[0] = None

            with tc.tile_pool(name="pp2", bufs=4, space="PSUM") as pp2:
                POOLS["proj"] = pp2
                pump(len(fillers))

        if debug:
            nc.sync.dma_start(dram["dbg_qT"].ap(), qT[:])
            nc.sync.dma_start(dram["dbg_kT"].ap(), kT[:])
            nc.sync.dma_start(dram["dbg_vaug"].ap(), vaug[:])
            nc.sync.dma_start(dram["dbg_oT"].ap(), oT[:])


def build(use_bias=False, debug=False):
    nc = bacc.Bacc("TRN2", target_bir_lowering=False, debug=False)
    dram = {
        "xT": nc.dram_tensor("xT", [128, DC, L], F16, kind="ExternalInput"),
        "wq": nc.dram_tensor("wq", [128, DC, 512], F16, kind="ExternalInput"),
        "wk": nc.dram_tensor("wk", [128, DC, 512], F16, kind="ExternalInput"),
        "wv": nc.dram_tensor("wv", [128, DC, 512], F16, kind="ExternalInput"),
        "wp": nc.dram_tensor("wp", [128, NJJ, 1024], F16, kind="ExternalInput"),
        "cos4": nc.dram_tensor("cos4", [128, L], F16, kind="ExternalInput"),
        "sin4": nc.dram_tensor("sin4", [128, L], F16, kind="ExternalInput"),
        "tri2": nc.dram_tensor("tri2", [128, 2, 128], F16, kind="ExternalInput"),
        "out": nc.dram_tensor("out", [L, D], F16, kind="ExternalOutput"),
    }
    if use_bias:
        dram["bq"] = nc.dram_tensor("bq", [128, NJJ], F32, kind="ExternalInput")
        dram["bk"] = nc.dram_tensor("bk", [128, NJJ], F32, kind="ExternalInput")
        dram["bv"] = nc.dram_tensor("bv", [1, 512], F16, kind="ExternalInput")
    if debug:
        dram["dbg_qT"] = nc.dram_tensor("dbg_qT", [128, NJJ, L], F16, kind="ExternalOutput")
        dram["dbg_kT"] = nc.dram_tensor("dbg_kT", [128, NJJ, L], F16, kind="ExternalOutput")
        dram["dbg_vaug"] = nc.dram_tensor("dbg_vaug", [128, NTT, HL, 128], F16, kind="ExternalOutput")
        dram["dbg_oT"] = nc.dram_tensor("dbg_oT", [128, NJJ, L], F16, kind="ExternalOutput")
    with tile.TileContext(nc) as tc:
        _emit(nc, tc, dram, use_bias, debug=debug)
    nc.compile()
    return nc


def host_inputs(x, W_qkv, b_qkv, W_proj, use_bias=False):
    """Build the 8 per-core input maps (numpy, fp16-cast, pre-laid-out)."""
    x = np.asarray(x, np.float32)
    W_qkv = np.asarray(W_qkv, np.float32)
    b_qkv = np.asarray(b_qkv, np.float32)
    W_proj = np.asarray(W_proj, np.float32)

    inv_freq = 1.0 / (ROPE_THETA ** (np.arange(0, Dh, 2, dtype=np.float32) / Dh))
    t = np.arange(L, dtype=np.float32)
    freqs = np.outer(t, inv_freq)            # [L, 32]
    cosT = np.cos(freqs).T.astype(np.float32)  # [32, L]
    sinT = np.sin(freqs).T.astype(np.float32)
    cos4 = np.tile(cosT, (4, 1)).astype(np.float16)           # [128, L]
    sin4 = np.concatenate([-sinT, sinT, -sinT, sinT], 0).astype(np.float16)

    # single causal triangle for the 128-col diagonal window, per head slot
    kk = np.arange(128)[:, None]
    jj_ = np.arange(128)[None, :]
    tri = (jj_ >= kk).astype(np.float16)       # [128, 128]
    tri2 = np.stack([tri, tri], axis=1)        # [128, 2, 128]

    perm = np.concatenate([np.arange(0, Dh, 2), np.arange(1, Dh, 2)])

    in_maps = []
    for c in range(N_CORES):
        b, g = c // 2, c % 2
        heads = np.arange(g * HL, g * HL + HL)
        qk_cols = np.concatenate([h * Dh + perm for h in heads])       # [512]
        v_lo = 2 * D + g * 512

        xT = np.ascontiguousarray(x[b].T)                  # [D, L]
        xT = xT.reshape(DC, 128, L).transpose(1, 0, 2)     # [128, DC, L]

        def wslice(cols_base, cols):
            w = W_qkv[:, cols_base + cols] if cols is not None \
                else W_qkv[:, cols_base:cols_base + 512]
            return np.ascontiguousarray(
                w.reshape(DC, 128, 512).transpose(1, 0, 2)).astype(np.float16)

        m = {
            "xT": xT.astype(np.float16),
            "wq": wslice(0, qk_cols),
            "wk": wslice(D, qk_cols),
            "wv": wslice(v_lo, None),
            "wp": np.ascontiguousarray(
                W_proj[g * 512:(g + 1) * 512, :]
                .reshape(NJJ, 128, 1024).transpose(1, 0, 2)).astype(np.float16),
            "cos4": cos4, "sin4": sin4, "tri2": tri2,
        }
        if use_bias:
            m["bq"] = np.ascontiguousarray(
                b_qkv[qk_cols].reshape(NJJ, 128).T).astype(np.float32)
            m["bk"] = np.ascontiguousarray(
                b_qkv[D + qk_cols].reshape(NJJ, 128).T).astype(np.float32)
            m["bv"] = b_qkv[v_lo:v_lo + 512].reshape(1, 512).astype(np.float16)
        in_maps.append(m)
    return in_maps


_NC = {}


def kernel(x, W_qkv, b_qkv, W_proj, b_proj, attention_mask):
    use_bias = bool(np.any(np.asarray(b_qkv)))
    nc = _NC.get(use_bias)
    if nc is None:
        nc = _NC[use_bias] = build(use_bias=use_bias)
    in_maps = host_inputs(x, W_qkv, b_qkv, W_proj, use_bias=use_bias)
    res = run_bass_kernel_spmd(nc, in_maps, core_ids=list(range(N_CORES)))
    b_proj = np.asarray(b_proj, np.float32)
    out = np.empty((B, L, D), np.float32)
    for b in range(B):
        out[b] = (res.results[2 * b]["out"].astype(np.float32)
                  + res.results[2 * b + 1]["out"].astype(np.float32)
                  + b_proj)
    return out


# revision 3
# speedup vs baseline: 1.0361x; 1.0361x over previous
"""Multi-head attention (RoPE, causal) Trainium2 kernel, 8-way sharded. v2.

Sharding: core c -> (batch b = c//2, head-group g = c%2 of 8 heads).
Each core computes its batch/head-group's attention output projected through
its W_proj row-slice; the host sums the two partial projections per batch
(fp16 partials, fp32 accumulate) and adds b_proj.

v2 changes vs baseline (all per-core):
  - exp fused across 2-ktile groups (4-bank PSUM pst) -> half the ACT
    instructions; diag groups run at reduced width.
  - S^T and P^T@V matmuls column-restricted on diagonal tiles.
  - causal mask = in-place multiply by a single [128,2,128] triangle on Pool
    (gpsimd), off ACT/DVE; no memsets (pst pre-zeroed once).
  - rope PSUM evacuation on DVE (plain copy; qkv bias folded in only when
    nonzero), v evacuation on DVE.
  - softmax reciprocal reads PSUM directly (no staging copy).
  - projection output staged to fp16, host sums partials in fp32.
  - software-pipelined emission: qk GEMMs for pair jj+1 and output-projection
    tiles are split into small "filler" units pumped into the PE queue
    between attention groups so the PE never idles while ACT runs exp.
"""

import numpy as np

import concourse.bass as bass
import concourse.bacc as bacc
import concourse.tile as tile
import concourse.mybir as mybir
from concourse.bass_utils import run_bass_kernel_spmd

F16 = mybir.dt.float16
F32 = mybir.dt.float32

B, L, D = 4, 2048, 1024
H, Dh = 16, 64
ROPE_THETA = 10000.0
N_CORES = 8
HL = 8           # heads per core
DC = D // 128    # 8 contraction chunks
NJJ = 4          # head pairs per core
NTC = L // 512   # 4 token chunks of 512
NTT = L // 128   # 16 token tiles of 128
NQC = L // 512   # 4 q chunks of 512


MM_LABELS = []


def _emit(nc, tc, dram, use_bias, debug=False):
    from contextlib import ExitStack

    _raw_mm = nc.tensor.matmul

    def _mm(label):
        MM_LABELS.append(label)
        return _raw_mm

    with ExitStack() as ctx:
        consts = ctx.enter_context(tc.tile_pool(name="consts", bufs=1))
        rope = ctx.enter_context(tc.tile_pool(name="rope", bufs=2))
        ptp = ctx.enter_context(tc.tile_pool(name="ptp", bufs=4))
        small = ctx.enter_context(tc.tile_pool(name="small", bufs=2))
        ostg = ctx.enter_context(tc.tile_pool(name="ostg", bufs=4))

        # ---- resident tensors -------------------------------------------
        xT = consts.tile([128, DC, L], F16)
        wq = consts.tile([128, DC, 512], F16)
        wk = consts.tile([128, DC, 512], F16)
        wv = consts.tile([128, DC, 512], F16)
        wp = consts.tile([128, NJJ, 1024], F16)
        cos4 = consts.tile([128, L], F16)
        sin4 = consts.tile([128, L], F16)
        tri2 = consts.tile([128, 2, 128], F16)
        qTZ = consts.tile([128, NJJ, 2, L], F16)
        kT = consts.tile([128, NJJ, L], F16)
        vaug = consts.tile([128, NTT, HL, 128], F16)
        oT = consts.tile([128, NJJ, L], F16)
        if use_bias:
            bq = consts.tile([128, NJJ], F32)
            bk = consts.tile([128, NJJ], F32)
            bv = consts.tile([1, 512], F16)
            ones1 = consts.tile([1, 128], F16)
            nc.sync.dma_start(bq[:], dram["bq"].ap())
            nc.sync.dma_start(bk[:], dram["bk"].ap())
            nc.sync.dma_start(bv[:], dram["bv"].ap())
            nc.vector.memset(ones1[:], 1.0)

        for dc in range(DC):
            nc.sync.dma_start(wq[:, dc, :], dram["wq"].ap()[:, dc, :])
            nc.sync.dma_start(xT[:, dc, :], dram["xT"].ap()[:, dc, :])
            nc.sync.dma_start(wk[:, dc, :], dram["wk"].ap()[:, dc, :])
            nc.sync.dma_start(wv[:, dc, :], dram["wv"].ap()[:, dc, :])
        nc.sync.dma_start(cos4[:], dram["cos4"].ap())
        nc.sync.dma_start(sin4[:], dram["sin4"].ap())
        nc.sync.dma_start(tri2[:], dram["tri2"].ap())
        nc.sync.dma_start(wp[:], dram["wp"].ap())
        nc.gpsimd.memset(vaug[:, :, :, 64:128], 1.0)
        nc.vector.memset(qTZ[:], 0.0)

        # ---- rope: psum -> qTZ/kT slice ---------------------------------
        def rope_store(ps, dstT, jj, ts, b_sb=None, bj=0):
            raw = rope.tile([128, 512], F16, tag="raw", name=f"raw_{jj}_{ts}")
            if b_sb is not None:
                nc.scalar.activation(
                    raw[:], ps[:], mybir.ActivationFunctionType.Identity,
                    bias=b_sb[:, bj:bj + 1],
                )
            else:
                nc.vector.tensor_copy(raw[:], ps[:])
            swp = rope.tile([128, 512], F16, tag="swp", name=f"swp_{jj}_{ts}")
            for blk in range(4):
                sb_ = (blk ^ 1) * 32
                nc.sync.dma_start(
                    swp[blk * 32:(blk + 1) * 32, :], raw[sb_:sb_ + 32, :]
                )
            t1 = rope.tile([128, 512], F16, tag="t1", name=f"t1_{jj}_{ts}")
            nc.vector.tensor_mul(t1[:], raw[:], cos4[:, ts:ts + 512])
            t2 = rope.tile([128, 512], F16, tag="t2", name=f"t2_{jj}_{ts}")
            nc.vector.tensor_mul(t2[:], swp[:], sin4[:, ts:ts + 512])
            if dstT is qTZ:
                nc.vector.tensor_add(qTZ[0:64, jj, 0, ts:ts + 512],
                                     t1[0:64, :], t2[0:64, :])
                nc.vector.tensor_add(qTZ[64:128, jj, 1, ts:ts + 512],
                                     t1[64:128, :], t2[64:128, :])
            else:
                nc.vector.tensor_add(dstT[:, jj, ts:ts + 512], t1[:], t2[:])

        with (
            tc.tile_pool(name="pgm", bufs=2, space="PSUM") as pgm,
        ):
            # ---- phase A: qk pair 0 + v, stream-consuming xT chunks ----
            def v_tile(tt, ps):
                for dc in range(DC):
                    _mm("A:v")(
                        ps[:],
                        xT[:, dc, tt * 128:(tt + 1) * 128],
                        wv[:, dc, :],
                        start=(dc == 0),
                        stop=(dc == DC - 1 and not use_bias),
                    )
                if use_bias:
                    _mm("A:vb")(
                        ps[:], ones1[:], bv[:], start=False, stop=True,
                    )

            def v_evac(tt, ps):
                nc.scalar.copy(
                    vaug[:, tt, :, 0:64],
                    ps[:].rearrange("p (h d) -> p h d", h=HL),
                )

            with tc.tile_pool(name="pga", bufs=1, space="PSUM") as pga, \
                 tc.tile_pool(name="pgk", bufs=1, space="PSUM") as pgk:
                psq = [pga.tile([128, 512], F32, tag=f"g{t}", name=f"q_{t}")
                       for t in range(NTC)]
                psk = [pgk.tile([128, 512], F32, tag=f"k{t}", name=f"k_{t}")
                       for t in range(2)]
                psv = [pgm.tile([128, 512], F32, tag="gemm", name=f"vps_{t}")
                       for t in range(2)]
                for dc in range(DC):
                    for tcn in range(NTC):
                        _mm("A:qk0")(
                            psq[tcn][:], wq[:, dc, 0:128],
                            xT[:, dc, tcn * 512:tcn * 512 + 512],
                            start=(dc == 0), stop=(dc == DC - 1),
                        )
                    for tcn in range(2):
                        _mm("A:qk0")(
                            psk[tcn][:], wk[:, dc, 0:128],
                            xT[:, dc, tcn * 512:tcn * 512 + 512],
                            start=(dc == 0), stop=(dc == DC - 1),
                        )
                    for vt in range(2):
                        _mm("A:v")(
                            psv[vt][:],
                            xT[:, dc, vt * 128:(vt + 1) * 128],
                            wv[:, dc, :],
                            start=(dc == 0),
                            stop=(dc == DC - 1 and not use_bias),
                        )
                bqs = bq if use_bias else None
                bks = bk if use_bias else None
                for tcn in range(NTC):
                    rope_store(psq[tcn], qTZ, 0, tcn * 512, b_sb=bqs, bj=0)
                if use_bias:
                    for vt in range(2):
                        _mm("A:vb")(
                            psv[vt][:], ones1[:], bv[:],
                            start=False, stop=True,
                        )
                for vt in range(2):
                    v_evac(vt, psv[vt])
                # k tcn 2,3 (xT fully resident now) into freed q banks
                for tcn in (2, 3):
                    ps = pga.tile([128, 512], F32, tag=f"g{tcn}",
                                  name=f"k_{tcn}")
                    for dc in range(DC):
                        _mm("A:qk0")(
                            ps[:], wk[:, dc, 0:128],
                            xT[:, dc, tcn * 512:tcn * 512 + 512],
                            start=(dc == 0), stop=(dc == DC - 1),
                        )
                    rope_store(ps, kT, 0, tcn * 512, b_sb=bks, bj=0)
                for tcn in range(2):
                    rope_store(psk[tcn], kT, 0, tcn * 512, b_sb=bks, bj=0)

                for tt in range(2, NTT):
                    ps = pgm.tile([128, 512], F32, tag="gemm",
                                  name=f"vps_{tt}")
                    v_tile(tt, ps)
                    v_evac(tt, ps)

            # ---- fillers ------------------------------------------------
            fillers = []
            POOLS = {"proj": pgm}

            def make_qk_units(jj):
                units = []
                for side, (w_sb, dstT) in enumerate(((wq, qTZ), (wk, kT))):
                    for tcn in range(NTC):
                        st = {}

                        def unit(k, w_sb=w_sb, dstT=dstT, tcn=tcn, st=st,
                                 side=side):
                            if k == 0:
                                st["ps"] = pgm.tile(
                                    [128, 512], F32, tag="gemm",
                                    name=f"qkps_{jj}_{side}_{tcn}")
                            ps = st["ps"]
                            for dc in (2 * k, 2 * k + 1):
                                _mm(f"F:qk{jj}")(
                                    ps[:],
                                    w_sb[:, dc, jj * 128:(jj + 1) * 128],
                                    xT[:, dc, tcn * 512:tcn * 512 + 512],
                                    start=(dc == 0),
                                    stop=(dc == DC - 1),
                                )
                            if k == 3:
                                b_sb = ((bq if side == 0 else bk)
                                        if use_bias else None)
                                rope_store(ps, dstT, jj, tcn * 512,
                                           b_sb=b_sb, bj=jj)

                        for k in range(4):
                            units.append((unit, k))
                return units

            def make_proj_units(qc):
                units = []
                for tt in range(4 * qc, 4 * qc + 4):
                    for cc in range(2):
                        def unit(k, tt=tt, cc=cc):
                            ps = POOLS["proj"].tile(
                                [128, 512], F32, tag="gemm",
                                name=f"projps_{tt}_{cc}")
                            for jj in range(NJJ):
                                _mm("F:proj")(
                                    ps[:],
                                    oT[:, jj, tt * 128:(tt + 1) * 128],
                                    wp[:, jj, cc * 512:cc * 512 + 512],
                                    start=(jj == 0), stop=(jj == NJJ - 1),
                                )
                            stage = ostg.tile([128, 512], F16, tag="stg",
                                              name=f"stg_{tt}_{cc}")
                            nc.vector.tensor_copy(stage[:], ps[:])
                            nc.sync.dma_start(
                                dram["out"].ap()[tt * 128:(tt + 1) * 128,
                                                 cc * 512:cc * 512 + 512],
                                stage[:],
                            )
                        units.append((unit, 0))
                return units

            def pump(n):
                for _ in range(n):
                    if not fillers:
                        return
                    f, k = fillers.pop(0)
                    f(k)

            # ---- attention ---------------------------------------------
            with (
                tc.tile_pool(name="pstp", bufs=2, space="PSUM") as pstp,
                tc.tile_pool(name="psop", bufs=1, space="PSUM") as psop,
            ):
                def emit_pv(batch, qc, jj, pso_t):
                    nkt = 4 * qc + 4
                    for ptile, kt in batch:
                        d = kt - 4 * qc
                        c0 = 128 * d if d > 0 else 0
                        for h in range(2):
                            _mm(f"PV:{jj}.{qc}")(
                                pso_t[:, h, c0:512],
                                vaug[:, kt, 2 * jj + h, :],
                                ptile[:, h, c0:512],
                                start=(kt == 0), stop=(kt == nkt - 1),
                                skip_group_check=True,
                            )

                pending_norm = [None]

                def normalize(jj, qc, pso_t):
                    qs = qc * 512
                    for h in range(2):
                        den = small.tile([64, 512], F32, tag="den",
                                         name=f"den_{jj}_{qc}_{h}")
                        nc.vector.tensor_copy(den[:], pso_t[64:128, h, :])
                        rec = small.tile([64, 512], F32, tag="rec",
                                         name=f"rec_{jj}_{qc}_{h}")
                        nc.vector.reciprocal_approx_fast(rec[:], den[:])
                        nc.vector.tensor_mul(
                            oT[64 * h:64 * h + 64, jj, qs:qs + 512],
                            pso_t[0:64, h, :], rec[:],
                        )
                    if jj == NJJ - 1:
                        fillers.extend(make_proj_units(qc))

                for jj in range(NJJ):
                    if jj < NJJ - 1:
                        fillers.extend(make_qk_units(jj + 1))
                    for qc in range(NQC):
                        qs = qc * 512
                        nbat = 2 * qc + 2
                        pso_t = psop.tile([128, 2, 512], F32, tag="pso",
                                          name=f"pso_{jj}_{qc}")
                        prevb = None
                        for g in range(nbat):
                            kts = (2 * g, 2 * g + 1)
                            psts = []
                            # both kts' S pairs back-to-back: one 64-row
                            # mode stretch on the PE
                            for kt in kts:
                                d = kt - 4 * qc
                                c0 = 128 * d if d > 0 else 0
                                ks = kt * 128
                                pst_t = pstp.tile(
                                    [128, 2, 512], F32, tag="pst",
                                    name=f"pst_{jj}_{qc}_{kt}")
                                for h in range(2):
                                    _mm(f"S:{jj}.{qc}")(
                                        pst_t[:, h, c0:512],
                                        kT[:, jj, ks:ks + 128],
                                        qTZ[:, jj, h,
                                            qs + c0:qs + 512],
                                        start=True, stop=True,
                                    )
                                psts.append((pst_t, kt, c0))
                            batch = []
                            for pst_t, kt, c0 in psts:
                                d = kt - 4 * qc
                                ptile = ptp.tile(
                                    [128, 2, 512], F16, tag="pt",
                                    name=f"pt_{jj}_{qc}_{kt}")
                                nc.scalar.activation(
                                    ptile[:, :, c0:512],
                                    pst_t[:, :, c0:512],
                                    mybir.ActivationFunctionType.Exp,
                                    scale=0.125,
                                )
                                if d >= 0:
                                    nc.gpsimd.tensor_mul(
                                        ptile[:, :, c0:c0 + 128],
                                        ptile[:, :, c0:c0 + 128],
                                        tri2[:],
                                    )
                                batch.append((ptile, kt))
                            if prevb is not None:
                                emit_pv(prevb, qc, jj, pso_t)
                            prevb = batch
                            pump(1)
                            if g == 0 and pending_norm[0] is not None:
                                normalize(*pending_norm[0])
                                pending_norm[0] = None
                        emit_pv(prevb, qc, jj, pso_t)
                        # normalize deferred one batch: keeps the DVE queue
                        # free of PV-dependent work ahead of filler evacs
                        pending_norm[0] = (jj, qc, pso_t)
                        pump(3)
                    # drain enough that next pair's qT/kT are ready
                    if jj < NJJ - 1:
                        pump(len(fillers))
                if pending_norm[0] is not None:
                    normalize(*pending_norm[0])
                    pending_norm[0] = None

            with tc.tile_pool(name="pp2", bufs=4, space="PSUM") as pp2:
                POOLS["proj"] = pp2
                pump(len(fillers))

        if debug:
            nc.sync.dma_start(dram["dbg_qT"].ap(), qTZ[:, :, 0, :])
            nc.sync.dma_start(dram["dbg_kT"].ap(), kT[:])
            nc.sync.dma_start(dram["dbg_vaug"].ap(), vaug[:])
            nc.sync.dma_start(dram["dbg_oT"].ap(), oT[:])


def build(use_bias=False, debug=False):
    nc = bacc.Bacc("TRN2", target_bir_lowering=False, debug=False)
    dram = {
        "xT": nc.dram_tensor("xT", [128, DC, L], F16, kind="ExternalInput"),
        "wq": nc.dram_tensor("wq", [128, DC, 512], F16, kind="ExternalInput"),
        "wk": nc.dram_tensor("wk", [128, DC, 512], F16, kind="ExternalInput"),
        "wv": nc.dram_tensor("wv", [128, DC, 512], F16, kind="ExternalInput"),
        "wp": nc.dram_tensor("wp", [128, NJJ, 1024], F16, kind="ExternalInput"),
        "cos4": nc.dram_tensor("cos4", [128, L], F16, kind="ExternalInput"),
        "sin4": nc.dram_tensor("sin4", [128, L], F16, kind="ExternalInput"),
        "tri2": nc.dram_tensor("tri2", [128, 2, 128], F16, kind="ExternalInput"),
        "out": nc.dram_tensor("out", [L, D], F16, kind="ExternalOutput"),
    }
    if use_bias:
        dram["bq"] = nc.dram_tensor("bq", [128, NJJ], F32, kind="ExternalInput")
        dram["bk"] = nc.dram_tensor("bk", [128, NJJ], F32, kind="ExternalInput")
        dram["bv"] = nc.dram_tensor("bv", [1, 512], F16, kind="ExternalInput")
    if debug:
        dram["dbg_qT"] = nc.dram_tensor("dbg_qT", [128, NJJ, L], F16, kind="ExternalOutput")
        dram["dbg_kT"] = nc.dram_tensor("dbg_kT", [128, NJJ, L], F16, kind="ExternalOutput")
        dram["dbg_vaug"] = nc.dram_tensor("dbg_vaug", [128, NTT, HL, 128], F16, kind="ExternalOutput")
        dram["dbg_oT"] = nc.dram_tensor("dbg_oT", [128, NJJ, L], F16, kind="ExternalOutput")
    with tile.TileContext(nc) as tc:
        _emit(nc, tc, dram, use_bias, debug=debug)
    nc.compile()
    return nc


def host_inputs(x, W_qkv, b_qkv, W_proj, use_bias=False):
    """Build the 8 per-core input maps (numpy, fp16-cast, pre-laid-out)."""
    x = np.asarray(x, np.float32)
    W_qkv = np.asarray(W_qkv, np.float32)
    b_qkv = np.asarray(b_qkv, np.float32)
    W_proj = np.asarray(W_proj, np.float32)

    inv_freq = 1.0 / (ROPE_THETA ** (np.arange(0, Dh, 2, dtype=np.float32) / Dh))
    t = np.arange(L, dtype=np.float32)
    freqs = np.outer(t, inv_freq)            # [L, 32]
    cosT = np.cos(freqs).T.astype(np.float32)  # [32, L]
    sinT = np.sin(freqs).T.astype(np.float32)
    cos4 = np.tile(cosT, (4, 1)).astype(np.float16)           # [128, L]
    sin4 = np.concatenate([-sinT, sinT, -sinT, sinT], 0).astype(np.float16)

    # single causal triangle for the 128-col diagonal window, per head slot
    kk = np.arange(128)[:, None]
    jj_ = np.arange(128)[None, :]
    tri = (jj_ >= kk).astype(np.float16)       # [128, 128]
    tri2 = np.stack([tri, tri], axis=1)        # [128, 2, 128]

    perm = np.concatenate([np.arange(0, Dh, 2), np.arange(1, Dh, 2)])

    in_maps = []
    for c in range(N_CORES):
        b, g = c // 2, c % 2
        heads = np.arange(g * HL, g * HL + HL)
        qk_cols = np.concatenate([h * Dh + perm for h in heads])       # [512]
        v_lo = 2 * D + g * 512

        xT = np.ascontiguousarray(x[b].T)                  # [D, L]
        xT = xT.reshape(DC, 128, L).transpose(1, 0, 2)     # [128, DC, L]

        def wslice(cols_base, cols):
            w = W_qkv[:, cols_base + cols] if cols is not None \
                else W_qkv[:, cols_base:cols_base + 512]
            return np.ascontiguousarray(
                w.reshape(DC, 128, 512).transpose(1, 0, 2)).astype(np.float16)

        m = {
            "xT": xT.astype(np.float16),
            "wq": wslice(0, qk_cols),
            "wk": wslice(D, qk_cols),
            "wv": wslice(v_lo, None),
            "wp": np.ascontiguousarray(
                W_proj[g * 512:(g + 1) * 512, :]
                .reshape(NJJ, 128, 1024).transpose(1, 0, 2)).astype(np.float16),
            "cos4": cos4, "sin4": sin4, "tri2": tri2,
        }
        if use_bias:
            m["bq"] = np.ascontiguousarray(
                b_qkv[qk_cols].reshape(NJJ, 128).T).astype(np.float32)
            m["bk"] = np.ascontiguousarray(
                b_qkv[D + qk_cols].reshape(NJJ, 128).T).astype(np.float32)
            m["bv"] = b_qkv[v_lo:v_lo + 512].reshape(1, 512).astype(np.float16)
        in_maps.append(m)
    return in_maps


_NC = {}


def kernel(x, W_qkv, b_qkv, W_proj, b_proj, attention_mask):
    use_bias = bool(np.any(np.asarray(b_qkv)))
    nc = _NC.get(use_bias)
    if nc is None:
        nc = _NC[use_bias] = build(use_bias=use_bias)
    in_maps = host_inputs(x, W_qkv, b_qkv, W_proj, use_bias=use_bias)
    res = run_bass_kernel_spmd(nc, in_maps, core_ids=list(range(N_CORES)))
    b_proj = np.asarray(b_proj, np.float32)
    out = np.empty((B, L, D), np.float32)
    for b in range(B):
        out[b] = (res.results[2 * b]["out"].astype(np.float32)
                  + res.results[2 * b + 1]["out"].astype(np.float32)
                  + b_proj)
    return out
